# revision 16
# baseline (speedup 1.0000x reference)
"""MoE (top-2, 8 experts) Trainium2 Bass kernel — expert-parallel across 8 NeuronCores.

Each core e holds expert e's FFN weights. Every core runs the (replicated) router in
fp32, compacts the indices of tokens routed to its expert via triangular-matmul prefix
sums + per-tile indirect-DMA scatters (HW indirect DMA supports one index per
partition), gathers those token rows, and runs the SwiGLU FFN on just its routed
tokens (fp32r stage 1, bf16 stage 2).
The host only re-layouts inputs (transposes / dtype views), scatters per-expert outputs
back to token order, and sums the 8 per-expert partial outputs.
"""

import numpy as np
import ml_dtypes

import concourse.bacc as bacc
import concourse.bass as bass
import concourse.tile as tile
from concourse import mybir
from concourse.bass import IndirectOffsetOnAxis
from concourse.bass_utils import run_bass_kernel_spmd

P = 128
F32 = mybir.dt.float32
F32R = mybir.dt.float32r
BF16 = mybir.dt.bfloat16
I32 = mybir.dt.int32
AF = mybir.ActivationFunctionType
OP = mybir.AluOpType

# Full-size problem config (matches the graded nn_MoE problem).
FULL = dict(N=4096, D=1024, HD=2048, E=8, K=2, C=1280, RTB=512, TB2=256)
BIG = float(2**20)


def build_program(N, D, HD, E, K, C, RTB, TB2):
    """Build the single-core SPMD program (same BIR for all 8 cores)."""
    NT = N // P    # token tiles
    DC = D // P    # contraction chunks over D
    MT = HD // P   # hd tiles
    GT = C // P    # routed-slot tiles
    NS = N // RTB  # router stripes
    JP = RTB // P  # token tiles per stripe
    NB = C // TB2  # FFN token blocks
    ND2 = D // 512 if D >= 512 else 1
    DW = D // ND2  # stage-2 output chunk width

    nc = bacc.Bacc("TRN2", target_bir_lowering=False, debug=False, num_devices=8)

    # ---- DRAM I/O ----
    xT = nc.dram_tensor("xT", [D, N], F32, kind="ExternalInput").ap()
    x = nc.dram_tensor("x", [N, D], F32, kind="ExternalInput").ap()
    WgT = nc.dram_tensor("WgT", [D, E], F32, kind="ExternalInput").ap()
    W1T = nc.dram_tensor("W1T", [D, HD], F32, kind="ExternalInput").ap()
    WgateT = nc.dram_tensor("WgateT", [D, HD], F32, kind="ExternalInput").ap()
    W2Tb = nc.dram_tensor("W2Tb", [HD, D], BF16, kind="ExternalInput").ap()
    onehot = nc.dram_tensor("onehot", [P, E], F32, kind="ExternalInput").ap()
    iota_nt = nc.dram_tensor("iota_nt", [P, NT], F32, kind="ExternalInput").ap()
    tri = nc.dram_tensor("tri", [P, P], F32, kind="ExternalInput").ap()
    stri = nc.dram_tensor("stri", [NT, NT], F32, kind="ExternalInput").ap()
    ones1 = nc.dram_tensor("ones1", [1, P], F32, kind="ExternalInput").ap()
    onesc = nc.dram_tensor("onesc", [P, 1], F32, kind="ExternalInput").ap()
    elast = nc.dram_tensor("elast", [P, 1], F32, kind="ExternalInput").ap()
    ident = nc.dram_tensor("ident", [P, P], F32, kind="ExternalInput").ap()

    yg = nc.dram_tensor("yg", [C, D], F32, kind="ExternalOutput").ap()
    table = nc.dram_tensor("table", [C + P, 2], F32, kind="ExternalOutput").ap()
    loss = nc.dram_tensor("loss", [1, 1], F32, kind="ExternalOutput").ap()
    counts = nc.dram_tensor("counts", [1, E], F32, kind="ExternalOutput").ap()

    with tile.TileContext(nc) as tc:
        with (
            tc.tile_pool(name="consts", bufs=1) as cpool,
            tc.tile_pool(name="w2", bufs=1) as w2pool,
            tc.tile_pool(name="routing", bufs=1) as rpers,
            tc.tile_pool(name="act", bufs=1) as apool,
            tc.tile_pool(name="gslot", bufs=1) as gspool,
            tc.tile_pool(name="psloss", bufs=1, space="PSUM") as psloss,
        ):
            # constants
            oh_sb = cpool.tile([P, E], F32, tag="oh")
            nc.sync.dma_start(oh_sb[:], onehot[:])
            iota_sb = cpool.tile([P, NT], F32, tag="iota")
            nc.sync.dma_start(iota_sb[:], iota_nt[:])
            tri_sb = cpool.tile([P, P], F32, tag="tri")
            nc.sync.dma_start(tri_sb[:], tri[:])
            stri_sb = cpool.tile([NT, NT], F32, tag="stri")
            nc.sync.dma_start(stri_sb[:], stri[:])
            ones1_sb = cpool.tile([1, P], F32, tag="ones1")
            nc.sync.dma_start(ones1_sb[:], ones1[:])
            onesc_sb = cpool.tile([P, 1], F32, tag="onesc")
            nc.sync.dma_start(onesc_sb[:], onesc[:])
            elast_sb = cpool.tile([P, 1], F32, tag="elast")
            nc.sync.dma_start(elast_sb[:], elast[:])
            ident_sb = cpool.tile([P, P], F32, tag="ident")
            nc.sync.dma_start(ident_sb[:], ident[:])
            wg_sb = cpool.tile([P, DC, E], F32, tag="wg")
            nc.sync.dma_start(wg_sb[:], WgT.rearrange("(c p) e -> p c e", p=P))

            # routing persistents
            gate_all = rpers.tile([P, NT], F32, tag="gate_all")
            mask_all = rpers.tile([P, NT], F32, tag="mask_all")
            scan_sb = rpers.tile([P, NT], F32, tag="scan_sb")
            pos_f = rpers.tile([P, NT], F32, tag="pos_f")
            bigm = rpers.tile([P, NT], F32, tag="bigm")
            pos_i = rpers.tile([P, NT], I32, tag="pos_i")
            pay = rpers.tile([P, 2 * NT], F32, tag="pay")

            # act persistents (bf16) + gate-per-slot
            act_sb = []
            for m in range(MT):
                act_sb.append(apool.tile([P, C], BF16, tag=f"act_{m}", name=f"act_{m}"))
            gslot_all = gspool.tile([P, GT], F32, tag="gslot_all")

            loss_p = psloss.tile([1, E], F32, space="PSUM", tag="loss_p")
            loss_f = psloss.tile([1, E], F32, space="PSUM", tag="loss_f")

            # ---------------- Phase 1: router (fp32) ----------------
            with (
                tc.tile_pool(name="xt", bufs=2) as xtpool,
                tc.tile_pool(name="rt", bufs=3) as rt,
                tc.tile_pool(name="pslg", bufs=3, space="PSUM") as pslg,
                tc.tile_pool(name="psms", bufs=2, space="PSUM") as psms,
            ):
                for s in range(NS):
                    xs = xtpool.tile([P, DC, RTB], F32, tag="xs")
                    nc.sync.dma_start(
                        xs[:], xT[:, s * RTB:(s + 1) * RTB].rearrange("(c p) t -> p c t", p=P)
                    )
                    for jj in range(JP):
                        j = s * JP + jj
                        pl = pslg.tile([P, E], F32, space="PSUM", tag="pl")
                        for c in range(DC):
                            nc.tensor.matmul(
                                pl[:],
                                xs[:, c, jj * P:(jj + 1) * P],
                                wg_sb[:, c, :],
                                start=(c == 0),
                                stop=(c == DC - 1),
                            )
                        lg = rt.tile([P, E], F32, tag="lg")
                        nc.vector.tensor_copy(lg[:], pl[:])
                        srt = rt.tile([P, 8], F32, tag="srt")
                        nc.vector.max(srt[:], lg[:])
                        # logits are O(1); exp() is safe without max-subtraction
                        exps = rt.tile([P, E], F32, tag="exps")
                        nc.scalar.activation(exps[:], lg[:], AF.Exp)
                        e12 = rt.tile([P, 2], F32, tag="e12")
                        nc.scalar.activation(e12[:], srt[:, 0:2], AF.Exp)
                        z = rt.tile([P, 1], F32, tag="z")
                        nc.vector.reduce_sum(z[:], exps[:], axis=mybir.AxisListType.X)
                        invz = rt.tile([P, 1], F32, tag="invz")
                        nc.vector.reciprocal(invz[:], z[:])
                        den = rt.tile([P, 1], F32, tag="den")
                        nc.vector.tensor_tensor(out=den[:], in0=e12[:, 0:1], in1=e12[:, 1:2], op=OP.add)
                        invden = rt.tile([P, 1], F32, tag="invden")
                        nc.vector.reciprocal(invden[:], den[:])
                        mask8 = rt.tile([P, E], F32, tag="mask8")
                        nc.vector.tensor_tensor(
                            out=mask8[:], in0=lg[:], in1=srt[:, 1:2].to_broadcast([P, E]), op=OP.is_ge
                        )
                        # loss accumulators: sum_t probs = sum_t invz[t]*exps[t,:] via lhsT=invz
                        nc.tensor.matmul(
                            loss_p[:], invz[:], exps[:],
                            start=(j == 0), stop=(j == NT - 1), skip_group_check=True,
                        )
                        nc.tensor.matmul(
                            loss_f[:], onesc_sb[:], mask8[:],
                            start=(j == 0), stop=(j == NT - 1), skip_group_check=True,
                        )
                        # this core's gate column: invden * sum_e(exps * mask * onehot)
                        tt = rt.tile([P, E], F32, tag="tt")
                        nc.vector.tensor_tensor(out=tt[:], in0=exps[:], in1=oh_sb[:], op=OP.mult)
                        nc.vector.tensor_tensor(out=tt[:], in0=tt[:], in1=mask8[:], op=OP.mult)
                        gc = rt.tile([P, 1], F32, tag="gc")
                        nc.vector.reduce_sum(gc[:], tt[:], axis=mybir.AxisListType.X)
                        nc.vector.tensor_tensor(
                            out=gate_all[:, j:j + 1], in0=gc[:], in1=invden[:], op=OP.mult
                        )

                # ---------------- Phase 2: compaction ----------------
                nc.vector.tensor_scalar(
                    out=mask_all[:], in0=gate_all[:], scalar1=0.0, scalar2=None, op0=OP.is_gt
                )
                ps_scan = psms.tile([P, NT], F32, space="PSUM", tag="msc")
                nc.tensor.matmul(ps_scan[:], tri_sb[:], mask_all[:], start=True, stop=True)
                nc.vector.tensor_copy(scan_sb[:], ps_scan[:])
                ps_tot = psms.tile([1, NT], F32, space="PSUM", tag="msc")
                nc.tensor.matmul(ps_tot[:], onesc_sb[:], mask_all[:], start=True, stop=True)
                tot_sb = rt.tile([1, NT], F32, tag="tot_sb")
                nc.vector.tensor_copy(tot_sb[:], ps_tot[:])
                ps_totT = psms.tile([NT, 1], F32, space="PSUM", tag="msc")
                nc.tensor.transpose(ps_totT[:], tot_sb[:], ident_sb[:1, :1])
                totT_sb = rt.tile([NT, 1], F32, tag="totT_sb")
                nc.vector.tensor_copy(totT_sb[:], ps_totT[:])
                ps_offs = psms.tile([1, NT], F32, space="PSUM", tag="msc")
                nc.tensor.matmul(ps_offs[:], totT_sb[:], stri_sb[:], start=True, stop=True)
                offs_sb = rt.tile([1, NT], F32, tag="offs_sb")
                nc.vector.tensor_copy(offs_sb[:], ps_offs[:])
                ps_bc = psms.tile([P, NT], F32, space="PSUM", tag="msc")
                nc.tensor.matmul(ps_bc[:], ones1_sb[:], offs_sb[:], start=True, stop=True)
                # pos = (scan + offs - mask) for routed, BIG for unrouted
                nc.vector.tensor_tensor(out=pos_f[:], in0=scan_sb[:], in1=ps_bc[:], op=OP.add)
                nc.vector.tensor_tensor(out=pos_f[:], in0=pos_f[:], in1=mask_all[:], op=OP.subtract)
                trash = rt.tile([P, 1], F32, tag="trash")
                nc.vector.tensor_scalar_add(trash[:], iota_sb[:, 0:1], float(C))
                nc.vector.tensor_scalar(
                    out=bigm[:], in0=mask_all[:], scalar1=-1.0, scalar2=1.0, op0=OP.mult, op1=OP.add
                )
                nc.vector.tensor_tensor(
                    out=bigm[:], in0=bigm[:], in1=trash[:, 0:1].to_broadcast([P, NT]), op=OP.mult
                )
                nc.vector.tensor_tensor(out=pos_f[:], in0=pos_f[:], in1=mask_all[:], op=OP.mult)
                nc.vector.tensor_tensor(out=pos_f[:], in0=pos_f[:], in1=bigm[:], op=OP.add)
                nc.vector.tensor_copy(pos_i[:], pos_f[:])

                # ---------------- Phase 3: batched scatter [token_id, gate] ----------------
                zro = rt.tile([P, GT + 1, 2], F32, tag="zro")
                nc.vector.memset(zro[:], 0.0)
                nc.sync.dma_start(table.rearrange("(g p) i -> p g i", p=P), zro[:])
                nc.vector.tensor_copy(pay[:, 0:2 * NT:2], iota_sb[:])
                nc.vector.tensor_copy(pay[:, 1:2 * NT:2], gate_all[:])
                for j in range(NT):
                    nc.gpsimd.indirect_dma_start(
                        out=table[:, :],
                        out_offset=IndirectOffsetOnAxis(ap=pos_i[:, j:j + 1], axis=0),
                        in_=pay[:, 2 * j:2 * j + 2],
                        in_offset=None,
                    )

            # W2 (bf16) resident — traced after the router so its DMAs yield priority
            w2_sb = []
            for kk in range(MT):
                t = w2pool.tile([P, D], BF16, tag=f"w2_{kk}", name=f"w2_{kk}")
                nc.sync.dma_start(t[:], W2Tb[kk * P:(kk + 1) * P, :])
                w2_sb.append(t)

            # ---------------- Phases 4-5 under xTg scope ----------------
            with tc.tile_pool(name="xTg", bufs=1) as xtgpool:
                xTg_sb = []
                for c in range(DC):
                    xTg_sb.append(xtgpool.tile([P, C], F32R, tag=f"xTg_{c}", name=f"xTg_{c}"))

                # Phase 4: table readback, batched gathers, transpose
                with (
                    tc.tile_pool(name="gp", bufs=2) as gp,
                    tc.tile_pool(name="pstr", bufs=2, space="PSUM") as pstr,
                ):
                    tb_sb = gp.tile([P, GT, 2], F32, tag="tb", bufs=1)
                    nc.sync.dma_start(tb_sb[:], table[0:C, :].rearrange("(g p) i -> p g i", p=P))
                    idx_i = gp.tile([P, GT], I32, tag="idx", bufs=1)
                    nc.vector.tensor_copy(idx_i[:], tb_sb[:, :, 0])
                    nc.vector.tensor_copy(gslot_all[:], tb_sb[:, :, 1])
                    for g in range(GT):
                        xg = gp.tile([P, D], F32, tag="xg")
                        nc.gpsimd.indirect_dma_start(
                            out=xg[:, :],
                            out_offset=None,
                            in_=x[:, :],
                            in_offset=IndirectOffsetOnAxis(ap=idx_i[:, g:g + 1], axis=0),
                        )
                        for c in range(DC):
                            pt = pstr.tile([P, P], F32, space="PSUM", tag="pt")
                            nc.tensor.transpose(
                                pt[:], xg[:, c * P:(c + 1) * P], ident_sb[:],
                            )
                            nc.vector.tensor_copy(xTg_sb[c][:, g * P:(g + 1) * P], pt[:])

                # Phase 5: FFN stage 1 (fp32r) -> act (bf16)
                with (
                    tc.tile_pool(name="wp", bufs=3) as wp,
                    tc.tile_pool(name="s1t", bufs=2) as s1t,
                    tc.tile_pool(name="pss1", bufs=2, space="PSUM") as pss1,
                ):
                    for m in range(MT):
                        w1t = wp.tile([P, DC, P], F32R, tag="w1t")
                        nc.sync.dma_start(
                            w1t[:],
                            W1T[:, m * P:(m + 1) * P].rearrange("(c p) m -> p c m", p=P).bitcast(F32R),
                        )
                        wgt = wp.tile([P, DC, P], F32R, tag="wgt")
                        nc.sync.dma_start(
                            wgt[:],
                            WgateT[:, m * P:(m + 1) * P].rearrange("(c p) m -> p c m", p=P).bitcast(F32R),
                        )
                        for b in range(NB):
                            ph1 = pss1.tile([P, TB2], F32, space="PSUM", tag="ph1")
                            phg = pss1.tile([P, TB2], F32, space="PSUM", tag="phg")
                            for c in range(DC):
                                nc.tensor.matmul(
                                    ph1[:], w1t[:, c, :], xTg_sb[c][:, b * TB2:(b + 1) * TB2],
                                    start=(c == 0), stop=(c == DC - 1),
                                )
                            for c in range(DC):
                                nc.tensor.matmul(
                                    phg[:], wgt[:, c, :], xTg_sb[c][:, b * TB2:(b + 1) * TB2],
                                    start=(c == 0), stop=(c == DC - 1),
                                )
                            s1 = s1t.tile([P, TB2], F32, tag="s1")
                            nc.scalar.activation(s1[:], ph1[:], AF.Sigmoid)
                            nc.vector.tensor_tensor(out=s1[:], in0=s1[:], in1=ph1[:], op=OP.mult)
                            nc.vector.tensor_tensor(
                                out=act_sb[m][:, b * TB2:(b + 1) * TB2], in0=s1[:], in1=phg[:], op=OP.mult
                            )

            # ---------------- Phase 6: FFN stage 2 (bf16) ----------------
            with (
                tc.tile_pool(name="yp", bufs=2) as yp,
                tc.tile_pool(name="pss2", bufs=2, space="PSUM") as pss2,
            ):
                for g in range(GT):
                    ysb = yp.tile([P, D], F32, tag="ysb")
                    for n in range(ND2):
                        py = pss2.tile([P, DW], F32, space="PSUM", tag="py")
                        for kk in range(MT):
                            nc.tensor.matmul(
                                py[:], act_sb[kk][:, g * P:(g + 1) * P],
                                w2_sb[kk][:, n * DW:(n + 1) * DW],
                                start=(kk == 0), stop=(kk == MT - 1),
                            )
                        nc.vector.tensor_scalar(
                            out=ysb[:, n * DW:(n + 1) * DW], in0=py[:],
                            scalar1=gslot_all[:, g:g + 1], scalar2=None, op0=OP.mult,
                        )
                    nc.sync.dma_start(yg[g * P:(g + 1) * P, :], ysb[:])

                # ---------------- Phase 7: loss ----------------
                lp = yp.tile([1, E], F32, tag="lp")
                nc.vector.tensor_copy(lp[:], loss_p[:])
                lf = yp.tile([1, E], F32, tag="lf")
                nc.vector.tensor_copy(lf[:], loss_f[:])
                nc.sync.dma_start(counts[:, :], lf[:])
                pf = yp.tile([1, E], F32, tag="pf")
                nc.vector.tensor_tensor(out=pf[:], in0=lp[:], in1=lf[:], op=OP.mult)
                ls = yp.tile([1, 1], F32, tag="ls")
                nc.vector.reduce_sum(ls[:], pf[:], axis=mybir.AxisListType.X)
                ls2 = yp.tile([1, 1], F32, tag="ls2")
                nc.vector.tensor_scalar_mul(ls2[:], ls[:], 1.0 / (float(N) * float(N) * float(K)))
                nc.sync.dma_start(loss[:, :], ls2[:])

    nc.compile()
    return nc


def make_host_consts(N, E):
    NT = N // P
    tok = np.arange(N, dtype=np.float32).reshape(NT, P).T.copy()  # [P, NT], token id p+128j
    return {
        "iota_nt": tok,
        "tri": np.tril(np.ones((P, P), dtype=np.float32)).T.copy(),  # tri[k,m]=1 iff k<=m
        "stri": (np.arange(NT)[:, None] < np.arange(NT)[None, :]).astype(np.float32),
        "ones1": np.ones((1, P), dtype=np.float32),
        "onesc": np.ones((P, 1), dtype=np.float32),
        "elast": np.eye(P, dtype=np.float32)[:, P - 1:P].copy(),
        "ident": np.eye(P, dtype=np.float32),
    }


_PROGRAM_CACHE = {}


def _get_program(cfg):
    key = tuple(sorted(cfg.items()))
    if key not in _PROGRAM_CACHE:
        _PROGRAM_CACHE[key] = build_program(**cfg)
    return _PROGRAM_CACHE[key]


def make_in_maps(x2, Wg, W1, Wgate, W2, cfg):
    N, E = cfg["N"], cfg["E"]
    xT = np.ascontiguousarray(x2.T)
    WgT = np.ascontiguousarray(np.asarray(Wg, np.float32).T)
    consts = make_host_consts(N, E)
    in_maps = []
    for e in range(E):
        onehot_e = np.zeros((P, E), dtype=np.float32)
        onehot_e[:, e] = 1.0
        in_maps.append({
            "xT": xT,
            "x": x2,
            "WgT": WgT,
            "W1T": np.ascontiguousarray(np.asarray(W1[e], np.float32).T),
            "WgateT": np.ascontiguousarray(np.asarray(Wgate[e], np.float32).T),
            "W2Tb": np.ascontiguousarray(np.asarray(W2[e], np.float32).T).astype(ml_dtypes.bfloat16),
            "onehot": onehot_e,
            **consts,
        })
    return in_maps


def moe_run(x, Wg, W1, Wgate, W2, k, cfg=None, return_raw=False):
    cfg = dict(cfg or FULL)
    N, D, E, K, C = cfg["N"], cfg["D"], cfg["E"], cfg["K"], cfg["C"]
    assert int(k) == K, f"expected k={K}, got {k}"

    x = np.asarray(x, dtype=np.float32)
    B, S, Din = x.shape
    assert B * S == N and Din == D

    x2 = np.ascontiguousarray(x.reshape(N, D))
    nc = _get_program(cfg)
    in_maps = make_in_maps(x2, Wg, W1, Wgate, W2, cfg)
    res = run_bass_kernel_spmd(nc, in_maps, core_ids=list(range(E)), trace=False)

    out = np.zeros((N, D), dtype=np.float32)
    cts = res.results[0]["counts"].reshape(-1)
    if cts.max() > C:
        raise RuntimeError(f"expert capacity exceeded: counts={cts}, C={C}")
    for e in range(E):
        tbl = res.results[e]["table"][:cfg["C"]]
        gate = tbl[:, 1]
        m = gate > 0
        idx = tbl[m, 0].astype(np.int64)
        out[idx] += res.results[e]["yg"][m]
    loss = np.float32(res.results[0]["loss"][0, 0])
    moe_output = out.reshape(B, S, D)
    if return_raw:
        return (moe_output, loss), res
    return (moe_output, loss)


def kernel(x, Wg, W1, Wgate, W2, k):
    return moe_run(x, Wg, W1, Wgate, W2, k, cfg=FULL)


# revision 19
# speedup vs baseline: 1.0391x; 1.0391x over previous
"""MoE (top-2, 8 experts) Trainium2 Bass kernel — expert-parallel across 8 NeuronCores.

Each core e holds expert e's FFN weights. Every core runs the (replicated) router in
fp32, compacts the indices of tokens routed to its expert via triangular-matmul prefix
sums + per-tile indirect-DMA scatters (HW indirect DMA supports one index per
partition), gathers those token rows, and runs the SwiGLU FFN on just its routed
tokens (fp32r stage 1, bf16 stage 2).
The host only re-layouts inputs (transposes / dtype views), scatters per-expert outputs
back to token order, and sums the 8 per-expert partial outputs.
"""

import numpy as np
import ml_dtypes

import concourse.bacc as bacc
import concourse.bass as bass
import concourse.tile as tile
from concourse import mybir
from concourse.bass import IndirectOffsetOnAxis
from concourse.bass_utils import run_bass_kernel_spmd

P = 128
F32 = mybir.dt.float32
F32R = mybir.dt.float32r
BF16 = mybir.dt.bfloat16
I32 = mybir.dt.int32
AF = mybir.ActivationFunctionType
OP = mybir.AluOpType

# Full-size problem config (matches the graded nn_MoE problem).
FULL = dict(N=4096, D=1024, HD=2048, E=8, K=2, C=1280, RTB=512, TB2=256)
BIG = float(2**20)


def build_program(N, D, HD, E, K, C, RTB, TB2):
    """Build the single-core SPMD program (same BIR for all 8 cores)."""
    NT = N // P    # token tiles
    DC = D // P    # contraction chunks over D
    MT = HD // P   # hd tiles
    GT = C // P    # routed-slot tiles
    NS = N // RTB  # router stripes
    JP = RTB // P  # token tiles per stripe
    NB = C // TB2  # FFN token blocks
    ND2 = D // 512 if D >= 512 else 1
    DW = D // ND2  # stage-2 output chunk width

    nc = bacc.Bacc("TRN2", target_bir_lowering=False, debug=False, num_devices=8)

    # ---- DRAM I/O ----
    xT = nc.dram_tensor("xT", [D, N], F32, kind="ExternalInput").ap()
    x = nc.dram_tensor("x", [N, D], F32, kind="ExternalInput").ap()
    WgT = nc.dram_tensor("WgT", [D, E], F32, kind="ExternalInput").ap()
    W1T = nc.dram_tensor("W1T", [D, HD], F32, kind="ExternalInput").ap()
    WgateT = nc.dram_tensor("WgateT", [D, HD], F32, kind="ExternalInput").ap()
    W2Tb = nc.dram_tensor("W2Tb", [HD, D], BF16, kind="ExternalInput").ap()
    onehot = nc.dram_tensor("onehot", [P, E], F32, kind="ExternalInput").ap()
    iota_nt = nc.dram_tensor("iota_nt", [P, NT], F32, kind="ExternalInput").ap()
    tri = nc.dram_tensor("tri", [P, P], F32, kind="ExternalInput").ap()
    stri = nc.dram_tensor("stri", [NT, NT], F32, kind="ExternalInput").ap()
    ones1 = nc.dram_tensor("ones1", [1, P], F32, kind="ExternalInput").ap()
    onesc = nc.dram_tensor("onesc", [P, 1], F32, kind="ExternalInput").ap()
    elast = nc.dram_tensor("elast", [P, 1], F32, kind="ExternalInput").ap()
    ident = nc.dram_tensor("ident", [P, P], F32, kind="ExternalInput").ap()

    yg = nc.dram_tensor("yg", [C, D], F32, kind="ExternalOutput").ap()
    table = nc.dram_tensor("table", [C + P, 2], F32, kind="ExternalOutput").ap()
    loss = nc.dram_tensor("loss", [1, 1], F32, kind="ExternalOutput").ap()
    counts = nc.dram_tensor("counts", [1, E], F32, kind="ExternalOutput").ap()

    with tile.TileContext(nc) as tc:
        with (
            tc.tile_pool(name="consts", bufs=1) as cpool,
            tc.tile_pool(name="w2", bufs=1) as w2pool,
            tc.tile_pool(name="routing", bufs=1) as rpers,
            tc.tile_pool(name="act", bufs=1) as apool,
            tc.tile_pool(name="gslot", bufs=1) as gspool,
            tc.tile_pool(name="psloss", bufs=1, space="PSUM") as psloss,
        ):
            # constants
            oh_sb = cpool.tile([P, E], F32, tag="oh")
            nc.sync.dma_start(oh_sb[:], onehot[:])
            iota_sb = cpool.tile([P, NT], F32, tag="iota")
            nc.sync.dma_start(iota_sb[:], iota_nt[:])
            tri_sb = cpool.tile([P, P], F32, tag="tri")
            nc.sync.dma_start(tri_sb[:], tri[:])
            stri_sb = cpool.tile([NT, NT], F32, tag="stri")
            nc.sync.dma_start(stri_sb[:], stri[:])
            ones1_sb = cpool.tile([1, P], F32, tag="ones1")
            nc.sync.dma_start(ones1_sb[:], ones1[:])
            onesc_sb = cpool.tile([P, 1], F32, tag="onesc")
            nc.sync.dma_start(onesc_sb[:], onesc[:])
            elast_sb = cpool.tile([P, 1], F32, tag="elast")
            nc.sync.dma_start(elast_sb[:], elast[:])
            ident_sb = cpool.tile([P, P], F32, tag="ident")
            nc.sync.dma_start(ident_sb[:], ident[:])
            wg_sb = cpool.tile([P, DC, E], F32, tag="wg")
            nc.sync.dma_start(wg_sb[:], WgT.rearrange("(c p) e -> p c e", p=P))

            # routing persistents
            gate_all = rpers.tile([P, NT], F32, tag="gate_all")
            mask_all = rpers.tile([P, NT], F32, tag="mask_all")
            scan_sb = rpers.tile([P, NT], F32, tag="scan_sb")
            pos_f = rpers.tile([P, NT], F32, tag="pos_f")
            bigm = rpers.tile([P, NT], F32, tag="bigm")
            pos_i = rpers.tile([P, NT], I32, tag="pos_i")
            pay = rpers.tile([P, 2 * NT], F32, tag="pay")

            # act persistents (bf16) + gate-per-slot
            act_sb = []
            for m in range(MT):
                act_sb.append(apool.tile([P, C], BF16, tag=f"act_{m}", name=f"act_{m}"))
            gslot_all = gspool.tile([P, GT], F32, tag="gslot_all")

            loss_p = psloss.tile([1, E], F32, space="PSUM", tag="loss_p")
            loss_f = psloss.tile([1, E], F32, space="PSUM", tag="loss_f")

            # ---------------- Phase 1: router (fp32) ----------------
            with (
                tc.tile_pool(name="xt", bufs=2) as xtpool,
                tc.tile_pool(name="rt", bufs=3) as rt,
                tc.tile_pool(name="pslg", bufs=3, space="PSUM") as pslg,
                tc.tile_pool(name="psms", bufs=2, space="PSUM") as psms,
            ):
                # table zeroing up front (cheap, off the critical path)
                zro = rt.tile([P, GT + 1, 2], F32, tag="zro")
                nc.vector.memset(zro[:], 0.0)
                nc.sync.dma_start(table.rearrange("(g p) i -> p g i", p=P), zro[:])
                trash = rt.tile([P, 1], F32, tag="trash")
                nc.vector.tensor_scalar_add(trash[:], iota_sb[:, 0:1], float(C))

                # two halves: half h's scatters overlap half h+1's router on the PE
                HV = 2 if (NS % 2 == 0 and NT % 2 == 0 and NS >= 2) else 1
                NH = NT // HV
                SH = NS // HV
                grand0 = rt.tile([1, 1], F32, tag="grand0", bufs=1)
                for h in range(HV):
                    for s_ in range(h * SH, (h + 1) * SH):
                        xs = xtpool.tile([P, DC, RTB], F32, tag="xs")
                        nc.sync.dma_start(
                            xs[:], xT[:, s_ * RTB:(s_ + 1) * RTB].rearrange("(c p) t -> p c t", p=P)
                        )
                        for jj in range(JP):
                            j = s_ * JP + jj
                            pl = pslg.tile([P, E], F32, space="PSUM", tag="pl")
                            for c in range(DC):
                                nc.tensor.matmul(
                                    pl[:],
                                    xs[:, c, jj * P:(jj + 1) * P],
                                    wg_sb[:, c, :],
                                    start=(c == 0),
                                    stop=(c == DC - 1),
                                )
                            lg = rt.tile([P, E], F32, tag="lg")
                            nc.vector.tensor_copy(lg[:], pl[:])
                            srt = rt.tile([P, 8], F32, tag="srt")
                            nc.vector.max(srt[:], lg[:])
                            # logits are O(1); exp() is safe without max-subtraction
                            exps = rt.tile([P, E], F32, tag="exps")
                            nc.scalar.activation(exps[:], lg[:], AF.Exp)
                            e12 = rt.tile([P, 2], F32, tag="e12")
                            nc.scalar.activation(e12[:], srt[:, 0:2], AF.Exp)
                            z = rt.tile([P, 1], F32, tag="z")
                            nc.vector.reduce_sum(z[:], exps[:], axis=mybir.AxisListType.X)
                            invz = rt.tile([P, 1], F32, tag="invz")
                            nc.vector.reciprocal(invz[:], z[:])
                            den = rt.tile([P, 1], F32, tag="den")
                            nc.vector.tensor_tensor(out=den[:], in0=e12[:, 0:1], in1=e12[:, 1:2], op=OP.add)
                            invden = rt.tile([P, 1], F32, tag="invden")
                            nc.vector.reciprocal(invden[:], den[:])
                            mask8 = rt.tile([P, E], F32, tag="mask8")
                            nc.vector.tensor_tensor(
                                out=mask8[:], in0=lg[:], in1=srt[:, 1:2].to_broadcast([P, E]), op=OP.is_ge
                            )
                            # loss accumulators: sum_t probs = sum_t invz[t]*exps[t,:]
                            nc.tensor.matmul(
                                loss_p[:], invz[:], exps[:],
                                start=(j == 0), stop=(j == NT - 1), skip_group_check=True,
                            )
                            nc.tensor.matmul(
                                loss_f[:], onesc_sb[:], mask8[:],
                                start=(j == 0), stop=(j == NT - 1), skip_group_check=True,
                            )
                            # this core's gate column: invden * sum_e(exps * mask * onehot)
                            tt = rt.tile([P, E], F32, tag="tt")
                            nc.vector.tensor_tensor(out=tt[:], in0=exps[:], in1=oh_sb[:], op=OP.mult)
                            nc.vector.tensor_tensor(out=tt[:], in0=tt[:], in1=mask8[:], op=OP.mult)
                            gc = rt.tile([P, 1], F32, tag="gc")
                            nc.vector.reduce_sum(gc[:], tt[:], axis=mybir.AxisListType.X)
                            nc.vector.tensor_tensor(
                                out=gate_all[:, j:j + 1], in0=gc[:], in1=invden[:], op=OP.mult
                            )

                    # ---- per-half compaction + scatter (overlaps next half's router) ----
                    hsl = slice(h * NH, (h + 1) * NH)
                    nc.vector.tensor_scalar(
                        out=mask_all[:, hsl], in0=gate_all[:, hsl], scalar1=0.0, scalar2=None, op0=OP.is_gt
                    )
                    ps_scan = psms.tile([P, NH], F32, space="PSUM", tag="msc")
                    nc.tensor.matmul(ps_scan[:], tri_sb[:], mask_all[:, hsl], start=True, stop=True)
                    nc.vector.tensor_copy(scan_sb[:, hsl], ps_scan[:])
                    ps_tot = psms.tile([1, NH], F32, space="PSUM", tag="msc")
                    nc.tensor.matmul(ps_tot[:], onesc_sb[:], mask_all[:, hsl], start=True, stop=True)
                    tot_sb = rt.tile([1, NH], F32, tag="tot_sb")
                    nc.vector.tensor_copy(tot_sb[:], ps_tot[:])
                    if h == 0:
                        nc.vector.reduce_sum(grand0[:], tot_sb[:], axis=mybir.AxisListType.X)
                    ps_totT = psms.tile([NH, 1], F32, space="PSUM", tag="msc")
                    nc.tensor.transpose(ps_totT[:], tot_sb[:], ident_sb[:1, :1])
                    totT_sb = rt.tile([NH, 1], F32, tag="totT_sb")
                    nc.vector.tensor_copy(totT_sb[:], ps_totT[:])
                    ps_offs = psms.tile([1, NH], F32, space="PSUM", tag="msc")
                    nc.tensor.matmul(ps_offs[:], totT_sb[:], stri_sb[0:NH, 0:NH], start=True, stop=True)
                    offs_sb = rt.tile([1, NH], F32, tag="offs_sb")
                    nc.vector.tensor_copy(offs_sb[:], ps_offs[:])
                    if h >= 1:
                        nc.vector.tensor_scalar_add(offs_sb[:], offs_sb[:], grand0[0:1, 0:1])
                    ps_bc = psms.tile([P, NH], F32, space="PSUM", tag="msc")
                    nc.tensor.matmul(ps_bc[:], ones1_sb[:], offs_sb[:], start=True, stop=True)
                    # pos = (scan + offs - mask) for routed, trash slot C+p for unrouted
                    nc.vector.tensor_tensor(out=pos_f[:, hsl], in0=scan_sb[:, hsl], in1=ps_bc[:], op=OP.add)
                    nc.vector.tensor_tensor(out=pos_f[:, hsl], in0=pos_f[:, hsl], in1=mask_all[:, hsl], op=OP.subtract)
                    nc.vector.tensor_scalar(
                        out=bigm[:, hsl], in0=mask_all[:, hsl], scalar1=-1.0, scalar2=1.0, op0=OP.mult, op1=OP.add
                    )
                    nc.vector.tensor_tensor(
                        out=bigm[:, hsl], in0=bigm[:, hsl], in1=trash[:, 0:1].to_broadcast([P, NH]), op=OP.mult
                    )
                    nc.vector.tensor_tensor(out=pos_f[:, hsl], in0=pos_f[:, hsl], in1=mask_all[:, hsl], op=OP.mult)
                    nc.vector.tensor_tensor(out=pos_f[:, hsl], in0=pos_f[:, hsl], in1=bigm[:, hsl], op=OP.add)
                    nc.vector.tensor_copy(pos_i[:, hsl], pos_f[:, hsl])
                    nc.vector.tensor_copy(pay[:, 2 * h * NH:2 * (h + 1) * NH:2], iota_sb[:, hsl])
                    nc.vector.tensor_copy(pay[:, 2 * h * NH + 1:2 * (h + 1) * NH:2], gate_all[:, hsl])
                    for j in range(h * NH, (h + 1) * NH):
                        nc.gpsimd.indirect_dma_start(
                            out=table[:, :],
                            out_offset=IndirectOffsetOnAxis(ap=pos_i[:, j:j + 1], axis=0),
                            in_=pay[:, 2 * j:2 * j + 2],
                            in_offset=None,
                        )

            # W2 (bf16) resident — traced after the router so its DMAs yield priority
            w2_sb = []
            for kk in range(MT):
                t = w2pool.tile([P, D], BF16, tag=f"w2_{kk}", name=f"w2_{kk}")
                nc.sync.dma_start(t[:], W2Tb[kk * P:(kk + 1) * P, :])
                w2_sb.append(t)

            # ---------------- Phases 4-5 under xTg scope ----------------
            with tc.tile_pool(name="xTg", bufs=1) as xtgpool:
                xTg_sb = []
                for c in range(DC):
                    xTg_sb.append(xtgpool.tile([P, C], F32R, tag=f"xTg_{c}", name=f"xTg_{c}"))

                # Phase 4: table readback, batched gathers, transpose
                with (
                    tc.tile_pool(name="gp", bufs=3) as gp,
                    tc.tile_pool(name="pstr", bufs=4, space="PSUM") as pstr,
                ):
                    tb_sb = gp.tile([P, GT, 2], F32, tag="tb", bufs=1)
                    nc.sync.dma_start(tb_sb[:], table[0:C, :].rearrange("(g p) i -> p g i", p=P))
                    idx_i = gp.tile([P, GT], I32, tag="idx", bufs=1)
                    nc.vector.tensor_copy(idx_i[:], tb_sb[:, :, 0])
                    nc.vector.tensor_copy(gslot_all[:], tb_sb[:, :, 1])
                    for g in range(GT):
                        xg = gp.tile([P, D], F32, tag="xg")
                        nc.gpsimd.indirect_dma_start(
                            out=xg[:, :],
                            out_offset=None,
                            in_=x[:, :],
                            in_offset=IndirectOffsetOnAxis(ap=idx_i[:, g:g + 1], axis=0),
                        )
                        for c in range(DC):
                            pt = pstr.tile([P, P], F32, space="PSUM", tag="pt")
                            nc.tensor.transpose(
                                pt[:], xg[:, c * P:(c + 1) * P], ident_sb[:],
                            )
                            nc.vector.tensor_copy(xTg_sb[c][:, g * P:(g + 1) * P], pt[:])

                # Phase 5: FFN stage 1 (fp32r) -> act (bf16)
                with (
                    tc.tile_pool(name="wp", bufs=3) as wp,
                    tc.tile_pool(name="s1t", bufs=2) as s1t,
                    tc.tile_pool(name="pss1", bufs=2, space="PSUM") as pss1,
                ):
                    for m in range(MT):
                        w1t = wp.tile([P, DC, P], F32R, tag="w1t")
                        nc.sync.dma_start(
                            w1t[:],
                            W1T[:, m * P:(m + 1) * P].rearrange("(c p) m -> p c m", p=P).bitcast(F32R),
                        )
                        wgt = wp.tile([P, DC, P], F32R, tag="wgt")
                        nc.sync.dma_start(
                            wgt[:],
                            WgateT[:, m * P:(m + 1) * P].rearrange("(c p) m -> p c m", p=P).bitcast(F32R),
                        )
                        for b in range(NB):
                            ph1 = pss1.tile([P, TB2], F32, space="PSUM", tag="ph1")
                            phg = pss1.tile([P, TB2], F32, space="PSUM", tag="phg")
                            for c in range(DC):
                                nc.tensor.matmul(
                                    ph1[:], w1t[:, c, :], xTg_sb[c][:, b * TB2:(b + 1) * TB2],
                                    start=(c == 0), stop=(c == DC - 1),
                                )
                            for c in range(DC):
                                nc.tensor.matmul(
                                    phg[:], wgt[:, c, :], xTg_sb[c][:, b * TB2:(b + 1) * TB2],
                                    start=(c == 0), stop=(c == DC - 1),
                                )
                            s1 = s1t.tile([P, TB2], F32, tag="s1")
                            nc.scalar.activation(s1[:], ph1[:], AF.Sigmoid)
                            nc.vector.tensor_tensor(out=s1[:], in0=s1[:], in1=ph1[:], op=OP.mult)
                            nc.vector.tensor_tensor(
                                out=act_sb[m][:, b * TB2:(b + 1) * TB2], in0=s1[:], in1=phg[:], op=OP.mult
                            )

            # ---------------- Phase 6: FFN stage 2 (bf16) ----------------
            with (
                tc.tile_pool(name="yp", bufs=2) as yp,
                tc.tile_pool(name="pss2", bufs=2, space="PSUM") as pss2,
            ):
                for g in range(GT):
                    ysb = yp.tile([P, D], F32, tag="ysb")
                    for n in range(ND2):
                        py = pss2.tile([P, DW], F32, space="PSUM", tag="py")
                        for kk in range(MT):
                            nc.tensor.matmul(
                                py[:], act_sb[kk][:, g * P:(g + 1) * P],
                                w2_sb[kk][:, n * DW:(n + 1) * DW],
                                start=(kk == 0), stop=(kk == MT - 1),
                            )
                        nc.vector.tensor_scalar(
                            out=ysb[:, n * DW:(n + 1) * DW], in0=py[:],
                            scalar1=gslot_all[:, g:g + 1], scalar2=None, op0=OP.mult,
                        )
                    nc.sync.dma_start(yg[g * P:(g + 1) * P, :], ysb[:])

                # ---------------- Phase 7: loss ----------------
                lp = yp.tile([1, E], F32, tag="lp")
                nc.vector.tensor_copy(lp[:], loss_p[:])
                lf = yp.tile([1, E], F32, tag="lf")
                nc.vector.tensor_copy(lf[:], loss_f[:])
                nc.sync.dma_start(counts[:, :], lf[:])
                pf = yp.tile([1, E], F32, tag="pf")
                nc.vector.tensor_tensor(out=pf[:], in0=lp[:], in1=lf[:], op=OP.mult)
                ls = yp.tile([1, 1], F32, tag="ls")
                nc.vector.reduce_sum(ls[:], pf[:], axis=mybir.AxisListType.X)
                ls2 = yp.tile([1, 1], F32, tag="ls2")
                nc.vector.tensor_scalar_mul(ls2[:], ls[:], 1.0 / (float(N) * float(N) * float(K)))
                nc.sync.dma_start(loss[:, :], ls2[:])

    nc.compile()
    return nc


def make_host_consts(N, E):
    NT = N // P
    tok = np.arange(N, dtype=np.float32).reshape(NT, P).T.copy()  # [P, NT], token id p+128j
    return {
        "iota_nt": tok,
        "tri": np.tril(np.ones((P, P), dtype=np.float32)).T.copy(),  # tri[k,m]=1 iff k<=m
        "stri": (np.arange(NT)[:, None] < np.arange(NT)[None, :]).astype(np.float32),
        "ones1": np.ones((1, P), dtype=np.float32),
        "onesc": np.ones((P, 1), dtype=np.float32),
        "elast": np.eye(P, dtype=np.float32)[:, P - 1:P].copy(),
        "ident": np.eye(P, dtype=np.float32),
    }


_PROGRAM_CACHE = {}


def _get_program(cfg):
    key = tuple(sorted(cfg.items()))
    if key not in _PROGRAM_CACHE:
        _PROGRAM_CACHE[key] = build_program(**cfg)
    return _PROGRAM_CACHE[key]


def make_in_maps(x2, Wg, W1, Wgate, W2, cfg):
    N, E = cfg["N"], cfg["E"]
    xT = np.ascontiguousarray(x2.T)
    WgT = np.ascontiguousarray(np.asarray(Wg, np.float32).T)
    consts = make_host_consts(N, E)
    in_maps = []
    for e in range(E):
        onehot_e = np.zeros((P, E), dtype=np.float32)
        onehot_e[:, e] = 1.0
        in_maps.append({
            "xT": xT,
            "x": x2,
            "WgT": WgT,
            "W1T": np.ascontiguousarray(np.asarray(W1[e], np.float32).T),
            "WgateT": np.ascontiguousarray(np.asarray(Wgate[e], np.float32).T),
            "W2Tb": np.ascontiguousarray(np.asarray(W2[e], np.float32).T).astype(ml_dtypes.bfloat16),
            "onehot": onehot_e,
            **consts,
        })
    return in_maps


def moe_run(x, Wg, W1, Wgate, W2, k, cfg=None, return_raw=False):
    cfg = dict(cfg or FULL)
    N, D, E, K, C = cfg["N"], cfg["D"], cfg["E"], cfg["K"], cfg["C"]
    assert int(k) == K, f"expected k={K}, got {k}"

    x = np.asarray(x, dtype=np.float32)
    B, S, Din = x.shape
    assert B * S == N and Din == D

    x2 = np.ascontiguousarray(x.reshape(N, D))
    nc = _get_program(cfg)
    in_maps = make_in_maps(x2, Wg, W1, Wgate, W2, cfg)
    res = run_bass_kernel_spmd(nc, in_maps, core_ids=list(range(E)), trace=False)

    out = np.zeros((N, D), dtype=np.float32)
    cts = res.results[0]["counts"].reshape(-1)
    if cts.max() > C:
        raise RuntimeError(f"expert capacity exceeded: counts={cts}, C={C}")
    for e in range(E):
        tbl = res.results[e]["table"][:cfg["C"]]
        gate = tbl[:, 1]
        m = gate > 0
        idx = tbl[m, 0].astype(np.int64)
        out[idx] += res.results[e]["yg"][m]
    loss = np.float32(res.results[0]["loss"][0, 0])
    moe_output = out.reshape(B, S, D)
    if return_raw:
        return (moe_output, loss), res
    return (moe_output, loss)


def kernel(x, Wg, W1, Wgate, W2, k):
    return moe_run(x, Wg, W1, Wgate, W2, k, cfg=FULL)
